# revision 18
# baseline (speedup 1.0000x reference)
"""Trainium2 Bass kernel for 3x3 conv (stride 1, pad 1) + bias.

Problem: x (32,128,56,56) f32, weights (256,128,3,3) f32, bias (256,) f32
         -> out (32,256,56,56) f32.

Strategy: data-parallel over batch (4 images per core, 8 cores).
Per core: implicit GEMM. C_in=128 lives on the SBUF partition axis (the
matmul contraction dim). Each image is stored width+height zero-padded
(58x58 grid) in a flat per-image slot so that, for every 3x3 tap (kh,kw),
the conv becomes ONE shifted contiguous matmul over 8 output rows
(N = 8*58 = 464) accumulated in PSUM across the 9 taps. C_out=256 is
split into two 128-partition halves (the matmul M dim). Bias is added
during PSUM->SBUF eviction on the scalar engine.

Inputs are converted to bf16 on the host (fp32 matmul is 1/4 rate on
TRN2's PE; bf16 streams 1 row/cycle and accumulates in fp32 PSUM).
"""

import os
from contextlib import ExitStack

import ml_dtypes
import numpy as np

import concourse.bacc as bacc
import concourse.bass as bass
import concourse.mybir as mybir
import concourse.tile as tile
import concourse.bass_utils as bass_utils

N_CORES = 8
B, CIN, H, W = 32, 128, 56, 56
COUT = 256
BPC = B // N_CORES          # images per core
PW, PH = W + 1, H + 2       # grid 58 rows x 57 cols: one shared pad col
GRID = PW * PH              # 3306  (col 0 of each row is the zero pad;
                            #  col 57 === next row's col 0)
SLOT = GRID + 2             # +2 zero guard for the last row's col-57 read
                            #  (and the flat-slice bound of the 8x57 view)
RPC = 8                     # output rows per PSUM chunk
NCHUNK = H // RPC           # 7
NFREE = RPC * W             # 448 moving-dim elements per matmul (2D AP)
KK = 9                      # 3x3 taps

DT = mybir.dt.bfloat16
NPDT = ml_dtypes.bfloat16

_CACHE: dict = {}


def _build():
    """Build the per-core Bass program (same program on all 8 cores)."""
    nc = bacc.Bacc("TRN2", target_bir_lowering=False, debug=False,
                   num_devices=N_CORES)
    f32 = mybir.dt.float32
    xp = nc.dram_tensor("xp", [BPC, CIN, SLOT], DT, kind="ExternalInput").ap()
    wt = nc.dram_tensor("wt", [CIN, KK * COUT], DT, kind="ExternalInput").ap()
    b2 = nc.dram_tensor("b2", [2, 128, 1], f32, kind="ExternalInput").ap()
    out = nc.dram_tensor("out", [BPC, COUT, H, W], f32,
                         kind="ExternalOutput").ap()

    with tile.TileContext(nc) as tc, ExitStack() as ctx:
        const_pool = ctx.enter_context(tc.tile_pool(name="const", bufs=1))
        xpool = ctx.enter_context(tc.tile_pool(name="xp_pool", bufs=1))
        epool = ctx.enter_context(tc.tile_pool(name="epool", bufs=6))
        psum = ctx.enter_context(
            tc.tile_pool(name="psum", bufs=7, space="PSUM"))
        wupool = ctx.enter_context(
            tc.tile_pool(name="wupool", bufs=1, space="PSUM"))

        wbuf = const_pool.tile([CIN, KK * COUT], DT)
        xbuf = xpool.tile([CIN, BPC * SLOT], DT)
        bbuf = const_pool.tile([128, 2], f32)

        # HAM warmup: ~8 junk matmuls while the input DMAs are in flight,
        # so the PE clock-gate is at 8/8 (2.4 GHz) when real work arrives.
        wrm = const_pool.tile([128, 512], DT)
        nc.vector.memset(wrm[:], 0)
        wps = wupool.tile([128, 512], f32)
        for _ in range(8):
            nc.tensor.matmul(wps[:], wrm[:, :128], wrm[:],
                             start=True, stop=True)

        # DMA-in: weights + first piece of image 0 go on the Scalar HWDGE
        # queue, the bulk goes on the Sync queue, so the first chunk's
        # operands aren't stuck behind the whole input stream.
        q = SLOT // 4
        wsplit = 5 * COUT  # taps 0-4 on sync, taps 5-8 on scalar
        nc.scalar.dma_start(xbuf[:, :q], xp[0][:, :q])
        nc.scalar.dma_start(wbuf[:, wsplit:], wt[:, wsplit:])
        nc.sync.dma_start(wbuf[:, :wsplit], wt[:, :wsplit])
        for piece in range(1, 4):
            lo, hi = piece * q, (piece + 1) * q if piece < 3 else SLOT
            nc.sync.dma_start(xbuf[:, lo:hi], xp[0][:, lo:hi])
        for h in range(2):
            nc.sync.dma_start(bbuf[:, h:h + 1], b2[h])
        hs = SLOT // 2
        for n in range(1, BPC):
            for lo, hi in ((0, hs), (hs, SLOT)):
                nc.sync.dma_start(
                    xbuf[:, n * SLOT + lo:n * SLOT + hi],
                    xp[n][:, lo:hi])

        pss = [psum.tile([128, NFREE], f32, name=f"ps{i}", tag=f"ps{i}",
                         bufs=1)
               for i in range(NCHUNK)]
        evs = [epool.tile([128, RPC * W], f32, name=f"ev{i}", tag=f"ev{i}",
                          bufs=1)
               for i in range(6)]
        ichunk = 0
        for n in range(BPC):
            for h in range(2):
                for c in range(NCHUNK):
                    ps = pss[c]
                    for k in range(KK):
                        kh, kw = divmod(k, 3)
                        s = n * SLOT + PW * (RPC * c + kh) + kw
                        rhs = xbuf[:, s:s + RPC * PW].rearrange(
                            "p (r c) -> p r c", c=PW)[:, :, :W]
                        nc.tensor.matmul(
                            ps[:],
                            wbuf[:, k * COUT + h * 128:
                                 k * COUT + h * 128 + 128],
                            rhs,
                            start=(k == 0),
                            stop=(k == KK - 1),
                        )
                    ev = evs[ichunk % 6]
                    ichunk += 1
                    od = out[n, h * 128:(h + 1) * 128,
                             c * RPC:(c + 1) * RPC].rearrange(
                                 "c r w -> c (r w)")
                    if ichunk < 8 * NCHUNK:
                        nc.scalar.activation(
                            ev[:], ps[:],
                            mybir.ActivationFunctionType.Identity,
                            bias=bbuf[:, h:h + 1])
                        nc.scalar.dma_start(od, ev[:])
                    else:
                        # final chunk: split the eviction across two engines
                        # and push two half-DMAs on separate queues so the
                        # end-of-kernel dependency chain is as short as
                        # possible.
                        half = NFREE // 2
                        nc.scalar.activation(
                            ev[:, :half], ps[:, :half],
                            mybir.ActivationFunctionType.Identity,
                            bias=bbuf[:, h:h + 1])
                        nc.vector.tensor_scalar_add(
                            ev[:, half:], ps[:, half:], bbuf[:, h:h + 1])
                        nc.gpsimd.dma_start(od[:, :half], ev[:, :half])
                        nc.gpsimd.dma_start(od[:, half:], ev[:, half:])
    nc.compile()
    return nc


def _prep(x, weights, bias):
    """Host-side reshape/pad/cast into the device layouts."""
    xpad = np.zeros((B, CIN, SLOT), dtype=NPDT)
    grid = xpad[:, :, :GRID].reshape(B, CIN, PH, PW)
    # rows 1..56 hold the image; col 0 is the zero pad column (col 57 of a
    # row aliases the next row's col 0, so one pad column serves both edges)
    grid[:, :, 1:1 + H, 1:1 + W] = np.asarray(x).astype(NPDT)
    # weights (co, ci, kh, kw) -> (ci, kh*kw*co) flat
    wt = np.ascontiguousarray(
        np.asarray(weights).transpose(1, 2, 3, 0)).reshape(
            CIN, KK * COUT).astype(NPDT)
    b2 = np.asarray(bias).astype(np.float32).reshape(2, 128, 1)
    return xpad, wt, b2


def kernel(x, weights, bias):
    if "nc" not in _CACHE:
        _CACHE["nc"] = _build()
    nc = _CACHE["nc"]
    xpad, wt, b2 = _prep(x, weights, bias)
    in_maps = [
        {"xp": xpad[i * BPC:(i + 1) * BPC], "wt": wt, "b2": b2}
        for i in range(N_CORES)
    ]
    res = bass_utils.run_bass_kernel_spmd(
        nc, in_maps, core_ids=list(range(N_CORES)),
        trace=bool(int(os.environ.get("CONV_TRACE", "0"))),
    )
    if os.environ.get("CONV_TRACE"):
        _CACHE["last_result"] = res
    return np.concatenate([r["out"] for r in res.results], axis=0)


# revision 20
# speedup vs baseline: 1.0096x; 1.0096x over previous
"""Trainium2 Bass kernel for 3x3 conv (stride 1, pad 1) + bias.

Problem: x (32,128,56,56) f32, weights (256,128,3,3) f32, bias (256,) f32
         -> out (32,256,56,56) f32.

Strategy: data-parallel over batch (4 images per core, 8 cores).
Per core: implicit GEMM. C_in=128 lives on the SBUF partition axis (the
matmul contraction dim). Each image is stored width+height zero-padded
(58x58 grid) in a flat per-image slot so that, for every 3x3 tap (kh,kw),
the conv becomes ONE shifted contiguous matmul over 8 output rows
(N = 8*58 = 464) accumulated in PSUM across the 9 taps. C_out=256 is
split into two 128-partition halves (the matmul M dim). Bias is added
during PSUM->SBUF eviction on the scalar engine.

Inputs are converted to bf16 on the host (fp32 matmul is 1/4 rate on
TRN2's PE; bf16 streams 1 row/cycle and accumulates in fp32 PSUM).
"""

import os
from contextlib import ExitStack

import ml_dtypes
import numpy as np

import concourse.bacc as bacc
import concourse.bass as bass
import concourse.mybir as mybir
import concourse.tile as tile
import concourse.bass_utils as bass_utils

N_CORES = 8
B, CIN, H, W = 32, 128, 56, 56
COUT = 256
BPC = B // N_CORES          # images per core
PW, PH = W + 1, H + 2       # grid 58 rows x 57 cols: one shared pad col
GRID = PW * PH              # 3306  (col 0 of each row is the zero pad;
                            #  col 57 === next row's col 0)
SLOT = GRID + 2             # +2 zero guard for the last row's col-57 read
                            #  (and the flat-slice bound of the 8x57 view)
RPC = 8                     # output rows per PSUM chunk
NCHUNK = H // RPC           # 7
NFREE = RPC * W             # 448 moving-dim elements per matmul (2D AP)
KK = 9                      # 3x3 taps

DT = mybir.dt.bfloat16
NPDT = ml_dtypes.bfloat16

_CACHE: dict = {}


def _build():
    """Build the per-core Bass program (same program on all 8 cores)."""
    nc = bacc.Bacc("TRN2", target_bir_lowering=False, debug=False,
                   num_devices=N_CORES)
    f32 = mybir.dt.float32
    xp = nc.dram_tensor("xp", [BPC, CIN, SLOT], DT, kind="ExternalInput").ap()
    wt = nc.dram_tensor("wt", [CIN, KK * COUT], DT, kind="ExternalInput").ap()
    b2 = nc.dram_tensor("b2", [2, 128, 1], f32, kind="ExternalInput").ap()
    out = nc.dram_tensor("out", [BPC, COUT, H, W], f32,
                         kind="ExternalOutput").ap()

    with tile.TileContext(nc) as tc, ExitStack() as ctx:
        const_pool = ctx.enter_context(tc.tile_pool(name="const", bufs=1))
        xpool = ctx.enter_context(tc.tile_pool(name="xp_pool", bufs=1))
        epool = ctx.enter_context(tc.tile_pool(name="epool", bufs=6))
        psum = ctx.enter_context(
            tc.tile_pool(name="psum", bufs=7, space="PSUM"))
        wupool = ctx.enter_context(
            tc.tile_pool(name="wupool", bufs=1, space="PSUM"))

        wbuf = const_pool.tile([CIN, KK * COUT], DT)
        xbuf = xpool.tile([CIN, BPC * SLOT], DT)
        bbuf = const_pool.tile([128, 2], f32)

        # HAM warmup: ~8 junk matmuls while the input DMAs are in flight,
        # so the PE clock-gate is at 8/8 (2.4 GHz) when real work arrives.
        wrm = const_pool.tile([128, 512], DT)
        nc.gpsimd.memset(wrm[:], 0)
        wps = wupool.tile([128, 512], f32)
        for _ in range(8):
            nc.tensor.matmul(wps[:], wrm[:, :128], wrm[:],
                             start=True, stop=True)

        # DMA-in: weights + first piece of image 0 go on the Scalar HWDGE
        # queue, the bulk goes on the Sync queue, so the first chunk's
        # operands aren't stuck behind the whole input stream.
        q = SLOT // 4
        wsplit = 5 * COUT  # taps 0-4 on sync, taps 5-8 on scalar
        nc.scalar.dma_start(xbuf[:, :q], xp[0][:, :q])
        nc.scalar.dma_start(wbuf[:, wsplit:], wt[:, wsplit:])
        nc.sync.dma_start(wbuf[:, :wsplit], wt[:, :wsplit])
        for piece in range(1, 4):
            lo, hi = piece * q, (piece + 1) * q if piece < 3 else SLOT
            nc.sync.dma_start(xbuf[:, lo:hi], xp[0][:, lo:hi])
        for h in range(2):
            nc.sync.dma_start(bbuf[:, h:h + 1], b2[h])
        hs = SLOT // 2
        for n in range(1, BPC):
            for lo, hi in ((0, hs), (hs, SLOT)):
                nc.sync.dma_start(
                    xbuf[:, n * SLOT + lo:n * SLOT + hi],
                    xp[n][:, lo:hi])

        pss = [psum.tile([128, NFREE], f32, name=f"ps{i}", tag=f"ps{i}",
                         bufs=1)
               for i in range(NCHUNK)]
        evs = [epool.tile([128, RPC * W], f32, name=f"ev{i}", tag=f"ev{i}",
                          bufs=1)
               for i in range(6)]
        ichunk = 0
        for n in range(BPC):
            for h in range(2):
                for c in range(NCHUNK):
                    ps = pss[c]
                    for k in range(KK):
                        kh, kw = divmod(k, 3)
                        s = n * SLOT + PW * (RPC * c + kh) + kw
                        rhs = xbuf[:, s:s + RPC * PW].rearrange(
                            "p (r c) -> p r c", c=PW)[:, :, :W]
                        nc.tensor.matmul(
                            ps[:],
                            wbuf[:, k * COUT + h * 128:
                                 k * COUT + h * 128 + 128],
                            rhs,
                            start=(k == 0),
                            stop=(k == KK - 1),
                        )
                    ev = evs[ichunk % 6]
                    ichunk += 1
                    od = out[n, h * 128:(h + 1) * 128,
                             c * RPC:(c + 1) * RPC].rearrange(
                                 "c r w -> c (r w)")
                    if ichunk < 8 * NCHUNK:
                        nc.scalar.activation(
                            ev[:], ps[:],
                            mybir.ActivationFunctionType.Identity,
                            bias=bbuf[:, h:h + 1])
                        nc.scalar.dma_start(od, ev[:])
                    else:
                        # final chunk: split the eviction across two engines
                        # and push two half-DMAs on separate queues so the
                        # end-of-kernel dependency chain is as short as
                        # possible.
                        half = NFREE // 2
                        nc.scalar.activation(
                            ev[:, :half], ps[:, :half],
                            mybir.ActivationFunctionType.Identity,
                            bias=bbuf[:, h:h + 1])
                        nc.vector.tensor_scalar_add(
                            ev[:, half:], ps[:, half:], bbuf[:, h:h + 1])
                        nc.scalar.dma_start(od[:, :half], ev[:, :half])
                        nc.sync.dma_start(od[:, half:], ev[:, half:])
    nc.compile()
    return nc


def _prep(x, weights, bias):
    """Host-side reshape/pad/cast into the device layouts."""
    xpad = np.zeros((B, CIN, SLOT), dtype=NPDT)
    grid = xpad[:, :, :GRID].reshape(B, CIN, PH, PW)
    # rows 1..56 hold the image; col 0 is the zero pad column (col 57 of a
    # row aliases the next row's col 0, so one pad column serves both edges)
    grid[:, :, 1:1 + H, 1:1 + W] = np.asarray(x).astype(NPDT)
    # weights (co, ci, kh, kw) -> (ci, kh*kw*co) flat
    wt = np.ascontiguousarray(
        np.asarray(weights).transpose(1, 2, 3, 0)).reshape(
            CIN, KK * COUT).astype(NPDT)
    b2 = np.asarray(bias).astype(np.float32).reshape(2, 128, 1)
    return xpad, wt, b2


def kernel(x, weights, bias):
    if "nc" not in _CACHE:
        _CACHE["nc"] = _build()
    nc = _CACHE["nc"]
    xpad, wt, b2 = _prep(x, weights, bias)
    in_maps = [
        {"xp": xpad[i * BPC:(i + 1) * BPC], "wt": wt, "b2": b2}
        for i in range(N_CORES)
    ]
    res = bass_utils.run_bass_kernel_spmd(
        nc, in_maps, core_ids=list(range(N_CORES)),
        trace=bool(int(os.environ.get("CONV_TRACE", "0"))),
    )
    if os.environ.get("CONV_TRACE"):
        _CACHE["last_result"] = res
    return np.concatenate([r["out"] for r in res.results], axis=0)
